# revision 3
# baseline (speedup 1.0000x reference)
"""Expert-parallel MoE kernel for Trainium2 (8 NeuronCores), v3.

v3: phase C uses gpsimd dma_gather(transpose=True) on a host-provided
bf16 copy of x, which gathers the routed token rows AND delivers them
transposed (contraction on partitions) in one instruction — no PE
transposes, half the gather bytes. Slots are enumerated in
sparse_gather's 16-partition-minor order end to end.
"""

import os
import sys

try:
    import antenv.axon_hooks  # noqa: F401
except ImportError:
    import types

    _m = types.ModuleType("antenv.axon_hooks")
    _m._hook = None

    def _set(hook):
        _m._hook = hook

    def _get():
        return _m._hook

    _m.set_axon_ntff_profile_hook = _set
    _m.get_axon_ntff_profile_hook = _get
    sys.modules["antenv.axon_hooks"] = _m

import ml_dtypes
import numpy as np

import concourse.bacc as bacc
import concourse.bass as bass
import concourse.mybir as mybir
import concourse.tile as tile
from concourse.bass_utils import run_bass_kernel_spmd
from concourse.masks import make_identity

N, D, E = 8192, 2048, 8
NCORES = 8
NLOC = N // NCORES   # tokens per core (gating shard)
P = 128
C = NLOC // P        # gating chunks per core
KC = D // P          # contraction chunks
NBS = 512            # psum bank free size (f32)
NB = D // NBS
CAP = 2176           # padded routed-token capacity per expert
NT = CAP // P        # slot tiles (17)
FP = CAP // 16       # sparse_gather compact free size (136)
GTILES = [1, 4, 4, 4, 4]  # slot tiles per gather chunk

f32 = mybir.dt.float32
bf16 = mybir.dt.bfloat16
i16 = mybir.dt.int16
i32 = mybir.dt.int32
u32 = mybir.dt.uint32
Alu = mybir.AluOpType
Act = mybir.ActivationFunctionType
Axis = mybir.AxisListType

LAST_RESULT = None


def _build():
    nc = bacc.Bacc("TRN2", target_bir_lowering=False, debug=False,
                   num_devices=NCORES)
    xb = nc.dram_tensor("xb", [N, D], bf16, kind="ExternalInput").ap()
    xsT = nc.dram_tensor("xsT", [D, NLOC], f32, kind="ExternalInput").ap()
    Wg = nc.dram_tensor("Wg", [D, E], f32, kind="ExternalInput").ap()
    We = nc.dram_tensor("We", [D, D], bf16, kind="ExternalInput").ap()
    be = nc.dram_tensor("be", [1, D], bf16, kind="ExternalInput").ap()
    idx8 = nc.dram_tensor("idx8", [16, 1], i32, kind="ExternalInput").ap()
    out = nc.dram_tensor("out", [CAP, D], f32, kind="ExternalOutput").ap()
    meta_tok = nc.dram_tensor("meta_tok", [CAP, 1], f32,
                              kind="ExternalOutput").ap()
    nfound = nc.dram_tensor("nfound", [1, 1], u32,
                            kind="ExternalOutput").ap()

    with tile.TileContext(nc) as tc:
        with (
            tc.tile_pool(name="big", bufs=1) as big,
            tc.tile_pool(name="work", bufs=2) as work,
            tc.tile_pool(name="xt", bufs=2) as xtp,
            tc.tile_pool(name="opool", bufs=2) as opool,
            tc.tile_pool(name="psumM", bufs=4, space="PSUM") as psumM,
            tc.tile_pool(name="psumG", bufs=2, space="PSUM") as psumG,
            tc.tile_pool(name="psumA", bufs=1, space="PSUM") as psumA,
            tc.tile_pool(name="dram", bufs=1, space="DRAM") as dram,
            tc.tile_pool(name="ccout", bufs=1, space="DRAM") as ccout,
        ):
            # tiny dummy collective: pays the one-time cross-core BARRIER
            # (~45us) concurrently with weight loads instead of before the
            # first real AllGather
            dummy_sb = big.tile([2, 16], f32)
            nc.vector.memset(dummy_sb[:], 0.0)
            dummy_in = dram.tile([2, 16], f32)
            dummy_out = dram.tile([16, 16], f32)
            nc.sync.dma_start(out=dummy_in[:], in_=dummy_sb[:])
            nc.gpsimd.collective_compute(
                "AllGather", Alu.bypass,
                replica_groups=[list(range(NCORES))],
                ins=[dummy_in[:].opt()], outs=[dummy_out[:].opt()])

            ident = big.tile([P, P], f32)
            make_identity(nc, ident[:])
            ident8 = big.tile([E, E], f32)
            make_identity(nc, ident8[:])
            ones_row = big.tile([1, P], bf16)
            nc.vector.memset(ones_row[:], 1.0)

            # resident expert weight [128, KC, 2048] bf16 (8 MB)
            w_sb = big.tile([P, KC, D], bf16)
            for kc in range(KC):
                nc.scalar.dma_start(out=w_sb[:, kc, :],
                                    in_=We[kc * P:(kc + 1) * P, :])
            be_sb = big.tile([1, D], bf16)
            nc.scalar.dma_start(out=be_sb[:], in_=be[:, :])

            # gating weights in exact f32 (routing must match fp32 ref)
            wg_sb = big.tile([P, KC, E], f32)
            nc.sync.dma_start(
                out=wg_sb[:],
                in_=Wg[:, :].rearrange("(b a) c -> a b c", b=KC))

            # ---------------- phase A: gating on local shard -------------
            # gate logits computed TRANSPOSED: pgT[8, 512] = sum_kc
            # Wg_kc^T-free x xsT_kc (both direct loads, no PE transposes).
            # two 512-token slices; the first AllGather overlaps slice 1.
            coeffT = big.tile([E, NLOC], f32)
            ag_in = dram.tile([E, NLOC], f32)
            ag_out = ccout.tile([NCORES * E, NLOC], f32)
            HN = NLOC // 2
            KB = 8   # kc per gating DMA (2 MB loads)
            for sl in range(2):
                pgt = psumG.tile([E, HN], f32, tag="pgt")
                for kb in range(KC // KB):
                    xst = work.tile([P, KB, HN], f32, tag="xst")
                    nc.sync.dma_start(
                        out=xst[:],
                        in_=xsT[kb * KB * P:(kb + 1) * KB * P,
                                sl * HN:(sl + 1) * HN].rearrange(
                                    "(b a) c -> a b c", b=KB))
                    for k2 in range(KB):
                        kc = kb * KB + k2
                        nc.tensor.matmul(pgt[:],
                                         lhsT=wg_sb[:, kc, :],
                                         rhs=xst[:, k2, :],
                                         start=(kc == 0),
                                         stop=(kc == KC - 1))
                gt_sb = work.tile([E, HN], f32, tag="gt")
                nc.vector.tensor_copy(out=gt_sb[:], in_=pgt[:])
                for c in range(HN // P):
                    cs = slice(c * P, (c + 1) * P)
                    pt = psumA.tile([P, E], f32, tag="pt")
                    nc.tensor.transpose(out=pt[:], in_=gt_sb[:, cs],
                                        identity=ident8[:])
                    g = work.tile([P, E], f32, tag="g")
                    nc.vector.tensor_copy(out=g[:], in_=pt[:])

                    # top-2 softmax -> dense coeff row [P, E]
                    m1n = work.tile([P, 1], f32, tag="m1n")
                    nc.vector.tensor_reduce(out=m1n[:], in_=g[:], axis=Axis.X,
                                            op=Alu.max, negate=True)
                    ge1 = work.tile([P, E], f32, tag="ge1")
                    nc.vector.tensor_scalar(out=ge1[:], in0=g[:],
                                            scalar1=m1n[:, 0:1], scalar2=0.0,
                                            op0=Alu.add, op1=Alu.is_ge)
                    g2 = work.tile([P, E], f32, tag="g2")
                    nc.vector.scalar_tensor_tensor(out=g2[:], in0=ge1[:],
                                                   scalar=-1e30, in1=g[:],
                                                   op0=Alu.mult, op1=Alu.add)
                    m2n = work.tile([P, 1], f32, tag="m2n")
                    nc.vector.tensor_reduce(out=m2n[:], in_=g2[:], axis=Axis.X,
                                            op=Alu.max, negate=True)
                    mask2 = work.tile([P, E], f32, tag="mask2")
                    nc.vector.tensor_scalar(out=mask2[:], in0=g[:],
                                            scalar1=m2n[:, 0:1], scalar2=0.0,
                                            op0=Alu.add, op1=Alu.is_ge)
                    ex = work.tile([P, E], f32, tag="ex")
                    nc.scalar.activation(out=ex[:], in_=g[:], func=Act.Exp,
                                         bias=m1n[:, 0:1], scale=1.0)
                    masked = work.tile([P, E], f32, tag="masked")
                    nc.vector.tensor_tensor(out=masked[:], in0=ex[:],
                                            in1=mask2[:], op=Alu.mult)
                    z = work.tile([P, 1], f32, tag="z")
                    nc.vector.tensor_reduce(out=z[:], in_=masked[:],
                                            axis=Axis.X, op=Alu.add)
                    rz = work.tile([P, 1], f32, tag="rz")
                    nc.vector.reciprocal(out=rz[:], in_=z[:])
                    coeff_c = work.tile([P, E], f32, tag="coefc")
                    nc.vector.tensor_scalar_mul(out=coeff_c[:], in0=masked[:],
                                                scalar1=rz[:, 0:1])
                    pct = psumA.tile([E, P], f32, tag="pct")
                    nc.tensor.transpose(out=pct[:], in_=coeff_c[:],
                                        identity=ident[:])
                    nc.vector.tensor_copy(out=coeffT[:, sl * HN:][:, cs],
                                          in_=pct[:])
                nc.sync.dma_start(out=ag_in[:, sl * HN:(sl + 1) * HN],
                                  in_=coeffT[:, sl * HN:(sl + 1) * HN])

            # ---------------- phase B: allgather + compaction ------------
            nc.gpsimd.collective_compute(
                "AllGather", Alu.bypass,
                replica_groups=[list(range(NCORES))],
                ins=[ag_in[:].opt()], outs=[ag_out[:].opt()])

            # my expert's coeff for all tokens: [16, 512], token at (j, f)
            # is t = j*512 + f
            i16t = big.tile([16, 1], i32)
            nc.sync.dma_start(out=i16t[:], in_=idx8[:, :])
            w16 = big.tile([16, N // 16], f32)
            nc.gpsimd.indirect_dma_start(
                out=w16[:], out_offset=None,
                in_=ag_out[:, :].rearrange("a (b c) -> (a b) c", b=2),
                in_offset=bass.IndirectOffsetOnAxis(ap=i16t[:], axis=0))

            # masked tokid / coeff, compact via sparse_gather
            tid16 = big.tile([16, N // 16], i32)
            nc.gpsimd.iota(tid16[:], pattern=[[1, N // 16]], base=0,
                           channel_multiplier=N // 16)
            tid16f = big.tile([16, N // 16], f32)
            nc.vector.tensor_copy(out=tid16f[:], in_=tid16[:])
            # pack tokid+coeff into one value: routed -> tid + coeff
            # (coeff in (0,1), ~11 fraction bits at tid<8192), else -1.
            mask16 = big.tile([16, N // 16], f32)
            nc.vector.tensor_scalar(out=mask16[:], in0=w16[:], scalar1=0.0,
                                    scalar2=None, op0=Alu.is_gt)
            mcomb = big.tile([16, N // 16], f32)
            nc.vector.scalar_tensor_tensor(out=mcomb[:], in0=w16[:],
                                           scalar=1.0, in1=tid16f[:],
                                           op0=Alu.add, op1=Alu.add)
            nc.vector.tensor_tensor(out=mcomb[:], in0=mcomb[:],
                                    in1=mask16[:], op=Alu.mult)
            nc.vector.tensor_scalar(out=mcomb[:], in0=mcomb[:], scalar1=-1.0,
                                    scalar2=None, op0=Alu.add)

            ccomb = big.tile([16, FP], f32)
            nc.vector.memset(ccomb[:], 0.0)
            nfa = big.tile([1, 1], u32)
            nc.gpsimd.sparse_gather(out=ccomb[:], in_=mcomb[:],
                                    num_found=nfa[:])
            # gather indices: floor via (x - 0.4999) round-to-nearest on
            # int cast (coeff in [0.0025, 0.9975]), clamped to [0, N-1]
            cclamp = big.tile([16, FP], f32)
            nc.vector.tensor_scalar(out=cclamp[:], in0=ccomb[:],
                                    scalar1=0.4999, scalar2=None,
                                    op0=Alu.subtract)
            nc.vector.tensor_scalar(out=cclamp[:], in0=cclamp[:], scalar1=0.0,
                                    scalar2=float(N - 1), op0=Alu.max,
                                    op1=Alu.min)
            idxs16s = big.tile([16, FP], i16)
            nc.vector.tensor_copy(out=idxs16s[:], in_=cclamp[:])
            idxs16 = big.tile([P, FP], i16)
            for r in range(8):
                eng = (nc.sync, nc.scalar, nc.gpsimd)[r % 3]
                eng.dma_start(out=idxs16[r * 16:(r + 1) * 16, :],
                              in_=idxs16s[:])

            nc.scalar.dma_start(out=nfound[:, :], in_=nfa[:])
            # meta out: packed tid+coeff, p-major rows (host floors)
            nc.scalar.dma_start(
                out=meta_tok[:, :].rearrange("(a b) c -> a (b c)", b=FP),
                in_=ccomb[:])

            # combP[p, g] = packed(slot g*128+p) via p-major DRAM bounce
            comb_d = dram.tile([CAP, 1], f32)
            nc.sync.dma_start(
                out=comb_d[:, :].rearrange("(a b) c -> a (b c)", b=FP),
                in_=ccomb[:])
            combP = big.tile([P, NT], f32)
            cv = comb_d[:, :].rearrange("(pl g ph) c -> pl ph (g c)",
                                       pl=16, g=NT, ph=8)
            for ph in range(8):
                eng = nc.sync if ph % 2 == 0 else nc.scalar
                eng.dma_start(out=combP[ph * 16:(ph + 1) * 16, :],
                              in_=cv[:, ph, :])
            # split: coefP = combP - floor(combP)
            fl = big.tile([P, NT], f32)
            nc.vector.tensor_scalar(out=fl[:], in0=combP[:], scalar1=0.4999,
                                    scalar2=None, op0=Alu.subtract)
            fli = big.tile([P, NT], i32)
            nc.vector.tensor_copy(out=fli[:], in_=fl[:])
            nc.vector.tensor_copy(out=fl[:], in_=fli[:])
            coefP = big.tile([P, NT], f32)
            nc.vector.tensor_tensor(out=coefP[:], in0=combP[:], in1=fl[:],
                                    op=Alu.subtract)

            # ---------------- phase C: expert compute --------------------
            # out row j = slot j (minor order): view [g, p] -> p g d
            out3 = out[:, :].rearrange("(g p) d -> p g d", p=P)
            g0 = 0
            for gc, ntile in enumerate(GTILES):
                ipg = ntile * P
                xeT = xtp.tile([P, KC, ipg], bf16, tag=f"xet{ntile}")
                nc.gpsimd.dma_gather(
                    out_ap=xeT[:], in_ap=xb[:, :],
                    idxs_ap=idxs16[:, g0 * 8:g0 * 8 + ipg // 16],
                    num_idxs=ipg, num_idxs_reg=ipg, elem_size=D,
                    transpose=True)
                for tg in range(ntile):
                    g = g0 + tg
                    ts = slice(tg * P, (tg + 1) * P)
                    o_sb = opool.tile([P, D], f32, tag="osb")
                    for nb in range(NB):
                        ns = slice(nb * NBS, (nb + 1) * NBS)
                        pb = psumM.tile([P, NBS], f32, tag="psb")
                        for kc in range(KC):
                            nc.tensor.matmul(pb[:],
                                             lhsT=xeT[:, kc, ts],
                                             rhs=w_sb[:, kc, ns],
                                             start=(kc == 0), stop=False)
                        nc.tensor.matmul(pb[:],
                                         lhsT=ones_row[:],
                                         rhs=be_sb[:, ns],
                                         start=False, stop=True)
                        nc.vector.tensor_scalar_mul(
                            out=o_sb[:, ns], in0=pb[:],
                            scalar1=coefP[:, g:g + 1])
                        if g == NT - 1:
                            nc.sync.dma_start(out=out3[:, g, ns],
                                              in_=o_sb[:, ns])
                    if g < NT - 1:
                        nc.sync.dma_start(out=out3[:, g, :], in_=o_sb[:])
                g0 += ntile

    nc.compile()
    return nc


_NC_CACHE = None


def kernel(inputs: np.ndarray, Wg: np.ndarray, We: np.ndarray,
           be: np.ndarray) -> np.ndarray:
    global LAST_RESULT, _NC_CACHE
    inputs = np.ascontiguousarray(inputs, dtype=np.float32)
    Wg = np.ascontiguousarray(Wg, dtype=np.float32)
    We = np.ascontiguousarray(We, dtype=np.float32)
    be = np.ascontiguousarray(be, dtype=np.float32)

    if _NC_CACHE is None:
        _NC_CACHE = _build()
    nc = _NC_CACHE

    xb = inputs.astype(ml_dtypes.bfloat16)
    in_maps = []
    for i in range(NCORES):
        idx8 = np.array([[c * 16 + i * 2 + h] for c in range(NCORES)
                         for h in range(2)], dtype=np.int32)
        in_maps.append({
            "xb": xb,
            "xsT": np.ascontiguousarray(
                inputs[i * NLOC:(i + 1) * NLOC].T),
            "Wg": Wg,
            "We": We[i].astype(ml_dtypes.bfloat16),
            "be": be[i:i + 1].astype(ml_dtypes.bfloat16),
            "idx8": idx8,
        })
    res = run_bass_kernel_spmd(nc, in_maps, core_ids=list(range(NCORES)))
    LAST_RESULT = res

    OUT = np.zeros((N, D), dtype=np.float32)
    s = np.arange(CAP)
    # meta_tok row r holds compact slot s_r = (r % FP) * 16 + r // FP;
    # out row j holds slot j. tokid(slot s) = meta_tok[(s % 16) * FP + s//16]
    meta_row_of_slot = (s % 16) * FP + s // 16
    for i in range(NCORES):
        mt = np.nan_to_num(res.results[i]["meta_tok"][:, 0], nan=0.0,
                           posinf=0.0, neginf=0.0)
        ids = np.clip(np.floor(mt[meta_row_of_slot]).astype(np.int64),
                      0, N - 1)
        o = res.results[i]["out"]
        nf = min(int(res.results[i]["nfound"][0, 0]), CAP)
        OUT[ids[:nf]] += o[:nf]
    return OUT
